# revision 13
# baseline (speedup 1.0000x reference)
"""BoneLengthLoss Trainium2 kernel.

Full inputs: pose_3d_pred (524288, 37, 3) f32, pose_3d_ref same, valid_mask
(524288, 37) bool.  Output: scalar f32 = sum(sq_err * bone_valid) /
sum(bone_valid) over all (batch, bone) pairs.

Strategy: pure data-parallel over 8 NeuronCores (batch dim).  The host-side
shard step gathers both bone endpoints into a J-major bf16 layout
[J(2), pose(2), coord(3), r, bone(32)] per (tile, partition), so on device:

  - the 64-bone endpoint gather is already done: ONE contiguous 2x-mode DVE
    subtract produces all bone-difference vectors (vs 13 strided 1x gathers),
  - squares are a contiguous plane range split between ACT (5 planes) and
    DVE (1 plane), in place,
  - the coord-sums, sqrt, length-diff, masked square-accumulate all run on
    contiguous APs at 2x,
  - the mask arrives as bf16 endpoint pairs, so bone_valid + the valid
    count are ONE fused scalar_tensor_tensor (accum_out) at 2x.

bf16 halves HBM traffic (tolerance is 2e-2; measured error ~1e-4).  The
length-diff runs on GpSimd to keep DVE/ACT balanced; chunks are small
(R=32 rows/partition) with 3-deep tile pools so ~3 chunks pipeline across
engines.  Each core returns per-partition partial (num, den); the host sums
8x128 partials and divides.
"""

import sys

sys.path.insert(0, "/opt/trn_rl_repo")

import numpy as np

# ---- problem constants (hardcoded; kernel.py must be self-contained) ----
N_CORES = 8
BATCH = 524288
KP = 37  # keypoints
NB = 32  # bones
B_CORE = BATCH // N_CORES  # 65536
P = 128  # SBUF partitions
R = 32  # batch rows per partition per tile
T = B_CORE // (P * R)  # tiles per core (16)
RB = R * NB  # bone entries per partition per tile (1024)

BONES = np.array(
    [(1, 2), (1, 3), (1, 4), (2, 5), (3, 6), (11, 12), (11, 13), (12, 14),
     (13, 14), (14, 15), (15, 16), (16, 17), (12, 18), (18, 20), (20, 22),
     (13, 19), (19, 21), (21, 23), (16, 24), (16, 25), (24, 26), (25, 26),
     (24, 27), (27, 29), (29, 31), (25, 28), (28, 30), (30, 32), (17, 33),
     (33, 34), (34, 35), (35, 36)], dtype=np.int32)
J1 = BONES[:, 0]
J2 = BONES[:, 1]

# how many of the 6 (pose, coord) square-planes ACT takes; DVE takes the rest
ACT_PLANES = 3

_COMPILED = None


def _build(T=T):
    from concourse import bacc, tile
    import concourse.mybir as mybir

    f32 = mybir.dt.float32
    DT = mybir.dt.bfloat16

    nc = bacc.Bacc("TRN2", target_bir_lowering=False, debug=False)

    # pose: [J(2), g(2), c(3), r(R), b(32)] flattened per (tile, partition)
    pq_d = nc.dram_tensor("pq", [T, P, 2 * 2 * 3 * R * NB], DT, kind="ExternalInput")
    # mask endpoint pairs as bf16: [J(2), r(R), b(32)]
    mask_d = nc.dram_tensor("mask", [T, P, 2 * R * NB], DT, kind="ExternalInput")
    out_d = nc.dram_tensor("out", [P, 2], f32, kind="ExternalOutput")

    with tile.TileContext(nc) as tc:
        with (
            tc.tile_pool(name="io", bufs=3) as io_pool,
            tc.tile_pool(name="work", bufs=3) as work_pool,
            tc.tile_pool(name="acc", bufs=1) as acc_pool,
        ):
            # First DRAM tile split in 2 so the compute pipeline ramps early.
            SUB = 2
            Rs = R // SUB
            chunks = [(0, s * Rs, Rs) for s in range(SUB)]
            chunks += [(t, 0, R) for t in range(1, T)]
            NCH = len(chunks)

            numstrip = acc_pool.tile([P, NCH], f32)
            denstrip = acc_pool.tile([P, NCH], f32)

            # Software-pipelined emission: each engine executes its queue
            # in program order, so per-chunk emission would stall e.g. DVE
            # on ACT's square.  Emitting stage-skewed keeps every engine's
            # queue filled with ready work from different chunks.
            st = [dict() for _ in range(NCH)]

            def stage_a(ci):  # DMA in
                t, roff, R_ = chunks[ci]
                RB_ = R_ * NB
                HP = 6 * RB_
                pq = io_pool.tile([P, 2 * HP], DT, tag="pq")
                m2 = io_pool.tile([P, 2 * RB_], DT, tag="m2")
                st[ci].update(pq=pq, m2=m2, RB_=RB_, HP=HP)
                if R_ == R:
                    nc.gpsimd.dma_start(pq[:], pq_d[t])
                    nc.gpsimd.dma_start(m2[:], mask_d[t])
                else:
                    src = pq_d[t].rearrange(
                        "p (q r b) -> p q r b", q=12, r=R, b=NB
                    )[:, :, roff : roff + R_, :]
                    nc.gpsimd.dma_start(
                        pq.rearrange("p (q r b) -> p q r b", q=12, r=R_, b=NB),
                        src,
                    )
                    msrc = mask_d[t].rearrange(
                        "p (q r b) -> p q r b", q=2, r=R, b=NB
                    )[:, :, roff : roff + R_, :]
                    nc.gpsimd.dma_start(
                        m2.rearrange("p (q r b) -> p q r b", q=2, r=R_, b=NB),
                        msrc,
                    )

            def stage_b(ci):  # diff + squares + bone_valid/den
                s = st[ci]
                pq, m2, RB_, HP = s["pq"], s["m2"], s["RB_"], s["HP"]
                # D = X[j2] - X[j1], both poses, ONE contiguous 2x subtract.
                # Layout [g, c, r, b].
                D = work_pool.tile([P, HP], DT, tag="D")
                nc.vector.tensor_sub(D[:], pq[:, HP:], pq[:, :HP])
                # squares in place; ACT takes ACT_PLANES RB_-sized planes
                # (split per pose so g0 adds unblock earlier), DVE the rest
                split = ACT_PLANES * RB_
                g0p = min(3, ACT_PLANES) * RB_
                nc.scalar.activation(
                    D[:, :g0p], D[:, :g0p],
                    mybir.ActivationFunctionType.Square,
                )
                if g0p < split:
                    nc.scalar.activation(
                        D[:, g0p:split], D[:, g0p:split],
                        mybir.ActivationFunctionType.Square,
                    )
                if split < HP:
                    nc.vector.tensor_mul(D[:, split:], D[:, split:], D[:, split:])
                # bone_valid = mask[J1]*mask[J2] on GpSimd; den = sum(MV)
                # via an ACT Relu accumulate (Relu(MV)=MV for 0/1 values;
                # Relu is resident in every ACT table set)
                MV = work_pool.tile([P, RB_], DT, tag="MV", bufs=4)
                nc.gpsimd.tensor_mul(MV[:], m2[:, :RB_], m2[:, RB_:])
                JK = work_pool.tile([P, RB_], DT, tag="JK", bufs=2)
                nc.scalar.activation(
                    JK[:],
                    MV[:],
                    mybir.ActivationFunctionType.Relu,
                    accum_out=denstrip[:, ci : ci + 1],
                )
                s.update(D=D, MV=MV)

            def stage_c(ci):  # coord sums + sqrt + length diff
                s = st[ci]
                D, RB_ = s["D"], s["RB_"]
                L2 = work_pool.tile([P, 2 * RB_], DT, tag="L2")
                for g in range(2):
                    d0 = 3 * g * RB_
                    lo = g * RB_
                    nc.vector.tensor_add(
                        L2[:, lo : lo + RB_],
                        D[:, d0 : d0 + RB_],
                        D[:, d0 + RB_ : d0 + 2 * RB_],
                    )
                    nc.vector.tensor_add(
                        L2[:, lo : lo + RB_],
                        L2[:, lo : lo + RB_],
                        D[:, d0 + 2 * RB_ : d0 + 3 * RB_],
                    )
                nc.scalar.sqrt(L2[:], L2[:])
                # E = pred_len - ref_len; GpSimd keeps it off the hot engines
                nc.gpsimd.tensor_sub(L2[:, :RB_], L2[:, :RB_], L2[:, RB_:])
                s.update(L2=L2)

            def stage_d(ci):  # masked error + num accumulate
                s = st[ci]
                L2, MV, RB_ = s["L2"], s["MV"], s["RB_"]
                ME = L2[:, :RB_]
                nc.vector.tensor_mul(ME, ME, MV[:])
                nc.scalar.activation(
                    ME,
                    ME,
                    mybir.ActivationFunctionType.Square,
                    accum_out=numstrip[:, ci : ci + 1],
                )
                st[ci] = None  # free references

            for it in range(NCH + 3):
                if it < NCH:
                    stage_a(it)
                if 1 <= it < NCH + 1:
                    stage_b(it - 1)
                if 2 <= it < NCH + 2:
                    stage_c(it - 2)
                if it >= 3:
                    stage_d(it - 3)

            acc2 = acc_pool.tile([P, 2], f32)
            nc.vector.reduce_sum(acc2[:, 0:1], numstrip[:], axis=mybir.AxisListType.X)
            nc.vector.reduce_sum(acc2[:, 1:2], denstrip[:], axis=mybir.AxisListType.X)
            nc.gpsimd.dma_start(out_d[:], acc2[:])

    nc.compile()
    return nc


def _get_nc():
    global _COMPILED
    if _COMPILED is None:
        _COMPILED = _build()
    return _COMPILED


def _make_in_maps(pose_3d_pred, pose_3d_ref, valid_mask):
    import concourse.mybir as mybir

    bf16 = mybir.dt.np(mybir.dt.bfloat16)
    BJ = np.concatenate([J1, J2])  # J-major endpoint order (64)

    pred = np.asarray(pose_3d_pred, dtype=np.float32)
    ref = np.asarray(pose_3d_ref, dtype=np.float32)
    mask = np.asarray(valid_mask).astype(np.float32)

    # gather endpoints, cast, and lay out [core,T,P, J,g,c,R,b]
    ga = np.stack([pred[:, BJ, :], ref[:, BJ, :]])  # [g, B, 2J*32, c]
    ga = ga.astype(bf16)
    ga = ga.reshape(2, N_CORES, T, P, R, 2, NB, 3)
    ga = ga.transpose(1, 2, 3, 5, 0, 7, 4, 6)  # -> core,T,P,J,g,c,R,b
    ga = np.ascontiguousarray(ga).reshape(N_CORES, T, P, 2 * 2 * 3 * R * NB)

    mg = mask[:, BJ].astype(bf16)  # [B, 64]
    mg = mg.reshape(N_CORES, T, P, R, 2, NB).transpose(0, 1, 2, 4, 3, 5)
    mg = np.ascontiguousarray(mg).reshape(N_CORES, T, P, 2 * R * NB)

    return [{"pq": ga[c], "mask": mg[c]} for c in range(N_CORES)]


def kernel(pose_3d_pred, pose_3d_ref, valid_mask, _trace=False):
    from concourse.bass_utils import run_bass_kernel_spmd

    nc = _get_nc()
    in_maps = _make_in_maps(pose_3d_pred, pose_3d_ref, valid_mask)
    res = run_bass_kernel_spmd(nc, in_maps, list(range(N_CORES)), trace=_trace)
    num = 0.0
    den = 0.0
    for i in range(N_CORES):
        o = res.results[i]["out"].astype(np.float64)
        num += o[:, 0].sum()
        den += o[:, 1].sum()
    out = np.float32(num / den)
    if _trace:
        return out, res
    return out


# revision 16
# speedup vs baseline: 1.1551x; 1.1551x over previous
"""BoneLengthLoss Trainium2 kernel.

Full inputs: pose_3d_pred (524288, 37, 3) f32, pose_3d_ref same, valid_mask
(524288, 37) bool.  Output: scalar f32 = sum(sq_err * bone_valid) /
sum(bone_valid) over all (batch, bone) pairs.

Strategy: pure data-parallel over 8 NeuronCores (batch dim).  The host-side
shard step gathers both bone endpoints into a J-major bf16 layout
[J(2), pose(2), coord(3), r, bone(32)] per (tile, partition), so on device:

  - the 64-bone endpoint gather is already done: ONE contiguous 2x-mode DVE
    subtract produces all bone-difference vectors (vs 13 strided 1x gathers),
  - squares are a contiguous plane range split between ACT (5 planes) and
    DVE (1 plane), in place,
  - the coord-sums, sqrt, length-diff, masked square-accumulate all run on
    contiguous APs at 2x,
  - the mask arrives as bf16 endpoint pairs, so bone_valid + the valid
    count are ONE fused scalar_tensor_tensor (accum_out) at 2x.

bf16 halves HBM traffic (tolerance is 2e-2; measured error ~1e-4).  The
length-diff runs on GpSimd to keep DVE/ACT balanced; chunks are small
(R=32 rows/partition) with 3-deep tile pools so ~3 chunks pipeline across
engines.  Each core returns per-partition partial (num, den); the host sums
8x128 partials and divides.
"""

import sys

sys.path.insert(0, "/opt/trn_rl_repo")

import numpy as np

# ---- problem constants (hardcoded; kernel.py must be self-contained) ----
N_CORES = 8
BATCH = 524288
KP = 37  # keypoints
NB = 32  # bones
B_CORE = BATCH // N_CORES  # 65536
P = 128  # SBUF partitions
R = 32  # batch rows per partition per tile
T = B_CORE // (P * R)  # tiles per core (16)
RB = R * NB  # bone entries per partition per tile (1024)

BONES = np.array(
    [(1, 2), (1, 3), (1, 4), (2, 5), (3, 6), (11, 12), (11, 13), (12, 14),
     (13, 14), (14, 15), (15, 16), (16, 17), (12, 18), (18, 20), (20, 22),
     (13, 19), (19, 21), (21, 23), (16, 24), (16, 25), (24, 26), (25, 26),
     (24, 27), (27, 29), (29, 31), (25, 28), (28, 30), (30, 32), (17, 33),
     (33, 34), (34, 35), (35, 36)], dtype=np.int32)
J1 = BONES[:, 0]
J2 = BONES[:, 1]

# how many of the 6 (pose, coord) square-planes ACT takes; DVE takes the rest
ACT_PLANES = 4

_COMPILED = None


def _build(T=T):
    from concourse import bacc, tile
    import concourse.mybir as mybir

    f32 = mybir.dt.float32
    DT = mybir.dt.bfloat16

    nc = bacc.Bacc("TRN2", target_bir_lowering=False, debug=False)

    # pose: [J(2), g(2), c(3), r(R), b(32)] flattened per (tile, partition)
    pq_d = nc.dram_tensor("pq", [T, P, 2 * 2 * 3 * R * NB], DT, kind="ExternalInput")
    # mask endpoint pairs as bf16: [J(2), r(R), b(32)]
    mask_d = nc.dram_tensor("mask", [T, P, 2 * R * NB], DT, kind="ExternalInput")
    out_d = nc.dram_tensor("out", [P, 2], f32, kind="ExternalOutput")

    with tile.TileContext(nc) as tc:
        with (
            tc.tile_pool(name="io", bufs=3) as io_pool,
            tc.tile_pool(name="work", bufs=3) as work_pool,
            tc.tile_pool(name="acc", bufs=1) as acc_pool,
        ):
            # First DRAM tile split in 2 so the compute pipeline ramps early.
            SUB = 2
            Rs = R // SUB
            chunks = [(0, s * Rs, Rs) for s in range(SUB)]
            chunks += [(t, 0, R) for t in range(1, T)]
            NCH = len(chunks)

            numstrip = acc_pool.tile([P, NCH], f32)
            denstrip = acc_pool.tile([P, NCH], f32)

            # Software-pipelined emission: each engine executes its queue
            # in program order, so per-chunk emission would stall e.g. DVE
            # on ACT's square.  Emitting stage-skewed keeps every engine's
            # queue filled with ready work from different chunks.
            st = [dict() for _ in range(NCH)]

            def stage_a(ci):  # DMA in
                t, roff, R_ = chunks[ci]
                RB_ = R_ * NB
                HP = 6 * RB_
                pq = io_pool.tile([P, 2 * HP], DT, tag="pq")
                m2 = io_pool.tile([P, 2 * RB_], DT, tag="m2")
                st[ci].update(pq=pq, m2=m2, RB_=RB_, HP=HP)
                if R_ == R:
                    nc.gpsimd.dma_start(pq[:], pq_d[t])
                    nc.gpsimd.dma_start(m2[:], mask_d[t])
                else:
                    src = pq_d[t].rearrange(
                        "p (q r b) -> p q r b", q=12, r=R, b=NB
                    )[:, :, roff : roff + R_, :]
                    nc.gpsimd.dma_start(
                        pq.rearrange("p (q r b) -> p q r b", q=12, r=R_, b=NB),
                        src,
                    )
                    msrc = mask_d[t].rearrange(
                        "p (q r b) -> p q r b", q=2, r=R, b=NB
                    )[:, :, roff : roff + R_, :]
                    nc.gpsimd.dma_start(
                        m2.rearrange("p (q r b) -> p q r b", q=2, r=R_, b=NB),
                        msrc,
                    )

            def stage_b(ci):  # diff + squares + bone_valid/den
                s = st[ci]
                pq, m2, RB_, HP = s["pq"], s["m2"], s["RB_"], s["HP"]
                # D = X[j2] - X[j1], both poses, ONE contiguous 2x subtract.
                # Layout [g, c, r, b].
                D = work_pool.tile([P, HP], DT, tag="D")
                nc.vector.tensor_sub(D[:], pq[:, HP:], pq[:, :HP])
                # squares in place; ACT takes ACT_PLANES RB_-sized planes
                # (split per pose so g0 adds unblock earlier), DVE the rest
                split = ACT_PLANES * RB_
                g0p = min(3, ACT_PLANES) * RB_
                nc.scalar.activation(
                    D[:, :g0p], D[:, :g0p],
                    mybir.ActivationFunctionType.Square,
                )
                if g0p < split:
                    nc.scalar.activation(
                        D[:, g0p:split], D[:, g0p:split],
                        mybir.ActivationFunctionType.Square,
                    )
                if split < HP:
                    nc.vector.tensor_mul(D[:, split:], D[:, split:], D[:, split:])
                # bone_valid = mask[J1]*mask[J2]; den = sum(MV) via an ACT
                # Relu accumulate (Relu(MV)=MV for 0/1 values; Relu is
                # resident in every ACT table set).  NOTE: GpSimd elementwise
                # is avoided everywhere — it shares an SBUF port with DVE
                # and measurably inflates every DVE op when active.
                MV = work_pool.tile([P, RB_], DT, tag="MV", bufs=4)
                nc.vector.tensor_mul(MV[:], m2[:, :RB_], m2[:, RB_:])
                JK = work_pool.tile([P, RB_], DT, tag="JK", bufs=2)
                nc.scalar.activation(
                    JK[:],
                    MV[:],
                    mybir.ActivationFunctionType.Relu,
                    accum_out=denstrip[:, ci : ci + 1],
                )
                s.update(D=D, MV=MV)

            def stage_c(ci):  # coord sums + sqrt + length diff
                s = st[ci]
                D, RB_ = s["D"], s["RB_"]
                L2 = work_pool.tile([P, 2 * RB_], DT, tag="L2")
                for g in range(2):
                    d0 = 3 * g * RB_
                    lo = g * RB_
                    nc.vector.tensor_add(
                        L2[:, lo : lo + RB_],
                        D[:, d0 : d0 + RB_],
                        D[:, d0 + RB_ : d0 + 2 * RB_],
                    )
                    nc.vector.tensor_add(
                        L2[:, lo : lo + RB_],
                        L2[:, lo : lo + RB_],
                        D[:, d0 + 2 * RB_ : d0 + 3 * RB_],
                    )
                nc.scalar.sqrt(L2[:], L2[:])
                # E = pred_len - ref_len
                nc.vector.tensor_sub(L2[:, :RB_], L2[:, :RB_], L2[:, RB_:])
                s.update(L2=L2)

            def stage_d(ci):  # masked error + num accumulate
                s = st[ci]
                L2, MV, RB_ = s["L2"], s["MV"], s["RB_"]
                ME = L2[:, :RB_]
                nc.vector.tensor_mul(ME, ME, MV[:])
                nc.scalar.activation(
                    ME,
                    ME,
                    mybir.ActivationFunctionType.Square,
                    accum_out=numstrip[:, ci : ci + 1],
                )
                st[ci] = None  # free references

            for it in range(NCH + 3):
                if it < NCH:
                    stage_a(it)
                if 1 <= it < NCH + 1:
                    stage_b(it - 1)
                if 2 <= it < NCH + 2:
                    stage_c(it - 2)
                if it >= 3:
                    stage_d(it - 3)

            acc2 = acc_pool.tile([P, 2], f32)
            nc.vector.reduce_sum(acc2[:, 0:1], numstrip[:], axis=mybir.AxisListType.X)
            nc.vector.reduce_sum(acc2[:, 1:2], denstrip[:], axis=mybir.AxisListType.X)
            nc.gpsimd.dma_start(out_d[:], acc2[:])

    nc.compile()
    return nc


def _get_nc():
    global _COMPILED
    if _COMPILED is None:
        _COMPILED = _build()
    return _COMPILED


def _make_in_maps(pose_3d_pred, pose_3d_ref, valid_mask):
    import concourse.mybir as mybir

    bf16 = mybir.dt.np(mybir.dt.bfloat16)
    BJ = np.concatenate([J1, J2])  # J-major endpoint order (64)

    pred = np.asarray(pose_3d_pred, dtype=np.float32)
    ref = np.asarray(pose_3d_ref, dtype=np.float32)
    mask = np.asarray(valid_mask).astype(np.float32)

    # gather endpoints, cast, and lay out [core,T,P, J,g,c,R,b]
    ga = np.stack([pred[:, BJ, :], ref[:, BJ, :]])  # [g, B, 2J*32, c]
    ga = ga.astype(bf16)
    ga = ga.reshape(2, N_CORES, T, P, R, 2, NB, 3)
    ga = ga.transpose(1, 2, 3, 5, 0, 7, 4, 6)  # -> core,T,P,J,g,c,R,b
    ga = np.ascontiguousarray(ga).reshape(N_CORES, T, P, 2 * 2 * 3 * R * NB)

    mg = mask[:, BJ].astype(bf16)  # [B, 64]
    mg = mg.reshape(N_CORES, T, P, R, 2, NB).transpose(0, 1, 2, 4, 3, 5)
    mg = np.ascontiguousarray(mg).reshape(N_CORES, T, P, 2 * R * NB)

    return [{"pq": ga[c], "mask": mg[c]} for c in range(N_CORES)]


def kernel(pose_3d_pred, pose_3d_ref, valid_mask, _trace=False):
    from concourse.bass_utils import run_bass_kernel_spmd

    nc = _get_nc()
    in_maps = _make_in_maps(pose_3d_pred, pose_3d_ref, valid_mask)
    res = run_bass_kernel_spmd(nc, in_maps, list(range(N_CORES)), trace=_trace)
    num = 0.0
    den = 0.0
    for i in range(N_CORES):
        o = res.results[i]["out"].astype(np.float64)
        num += o[:, 0].sum()
        den += o[:, 1].sum()
    out = np.float32(num / den)
    if _trace:
        return out, res
    return out


# revision 22
# speedup vs baseline: 1.2152x; 1.0520x over previous
"""BoneLengthLoss Trainium2 kernel.

Full inputs: pose_3d_pred (524288, 37, 3) f32, pose_3d_ref same, valid_mask
(524288, 37) bool.  Output: scalar f32 = sum(sq_err * bone_valid) /
sum(bone_valid) over all (batch, bone) pairs.

Strategy: pure data-parallel over 8 NeuronCores (batch dim).  The host-side
shard step gathers both bone endpoints into a J-major bf16 layout
[J(2), pose(2), coord(3), r, bone(32)] per (tile, partition), so on device:

  - the 64-bone endpoint gather is already done: ONE contiguous 2x-mode DVE
    subtract produces all bone-difference vectors (vs 13 strided 1x gathers),
  - squares are a contiguous plane range split between ACT (5 planes) and
    DVE (1 plane), in place,
  - the coord-sums, sqrt, length-diff, masked square-accumulate all run on
    contiguous APs at 2x,
  - the mask arrives as bf16 endpoint pairs, so bone_valid + the valid
    count are ONE fused scalar_tensor_tensor (accum_out) at 2x.

bf16 halves HBM traffic (tolerance is 2e-2; measured error ~1e-4).  The
length-diff runs on GpSimd to keep DVE/ACT balanced; chunks are small
(R=32 rows/partition) with 3-deep tile pools so ~3 chunks pipeline across
engines.  Each core returns per-partition partial (num, den); the host sums
8x128 partials and divides.
"""

import sys

sys.path.insert(0, "/opt/trn_rl_repo")

import numpy as np

# ---- problem constants (hardcoded; kernel.py must be self-contained) ----
N_CORES = 8
BATCH = 524288
KP = 37  # keypoints
NB = 32  # bones
B_CORE = BATCH // N_CORES  # 65536
P = 128  # SBUF partitions
R = 32  # batch rows per partition per tile
T = B_CORE // (P * R)  # tiles per core (16)
RB = R * NB  # bone entries per partition per tile (1024)

BONES = np.array(
    [(1, 2), (1, 3), (1, 4), (2, 5), (3, 6), (11, 12), (11, 13), (12, 14),
     (13, 14), (14, 15), (15, 16), (16, 17), (12, 18), (18, 20), (20, 22),
     (13, 19), (19, 21), (21, 23), (16, 24), (16, 25), (24, 26), (25, 26),
     (24, 27), (27, 29), (29, 31), (25, 28), (28, 30), (30, 32), (17, 33),
     (33, 34), (34, 35), (35, 36)], dtype=np.int32)
J1 = BONES[:, 0]
J2 = BONES[:, 1]

# how many of the 6 (pose, coord) square-planes ACT takes; DVE takes the rest
ACT_PLANES = 5

_COMPILED = None


def _build(T=T):
    from concourse import bacc, tile
    import concourse.mybir as mybir

    f32 = mybir.dt.float32
    DT = mybir.dt.bfloat16

    nc = bacc.Bacc("TRN2", target_bir_lowering=False, debug=False)

    # pose: [J(2), g(2), c(3), r(R), b(32)] flattened per (tile, partition)
    pq_d = nc.dram_tensor("pq", [T, P, 2 * 2 * 3 * R * NB], DT, kind="ExternalInput")
    # mask endpoint pairs as bf16: [J(2), r(R), b(32)]
    mask_d = nc.dram_tensor("mask", [T, P, 2 * R * NB], DT, kind="ExternalInput")
    out_d = nc.dram_tensor("out", [P, 2], f32, kind="ExternalOutput")

    with tile.TileContext(nc) as tc:
        with (
            tc.tile_pool(name="io", bufs=3) as io_pool,
            tc.tile_pool(name="work", bufs=3) as work_pool,
            tc.tile_pool(name="acc", bufs=1) as acc_pool,
            tc.tile_pool(name="psacc", bufs=1, space="PSUM") as psum_pool,
        ):
            # First DRAM tile split in 2 so the compute pipeline ramps early.
            SUB = 2
            Rs = R // SUB
            chunks = [(0, s * Rs, Rs) for s in range(SUB)]
            chunks += [(t, 0, R) for t in range(1, T)]
            NCH = len(chunks)

            numstrip = acc_pool.tile([P, NCH], f32)
            # den accumulates on the (otherwise idle) TensorEngine: a
            # ones-vector matmul sums MV across partitions into PSUM, and
            # PSUM accumulation chains the sum across all chunks.  Two
            # halves because one PSUM bank holds 512 f32.
            ones = acc_pool.tile([P, 1], DT)
            nc.gpsimd.memset(ones[:], 1.0)
            den_ps0 = psum_pool.tile([1, RB // 2], f32, tag="den_ps0")
            den_ps1 = psum_pool.tile([1, RB // 2], f32, tag="den_ps1")
            den_ps = [den_ps0, den_ps1]
            nc.vector.memset(den_ps[0][:], 0.0)
            nc.vector.memset(den_ps[1][:], 0.0)

            # Software-pipelined emission: each engine executes its queue
            # in program order, so per-chunk emission would stall e.g. DVE
            # on ACT's square.  Emitting stage-skewed keeps every engine's
            # queue filled with ready work from different chunks.
            st = [dict() for _ in range(NCH)]

            def stage_a(ci):  # DMA in
                t, roff, R_ = chunks[ci]
                RB_ = R_ * NB
                HP = 6 * RB_
                pq = io_pool.tile([P, 2 * HP], DT, tag="pq")
                m2 = io_pool.tile([P, 2 * RB_], DT, tag="m2")
                st[ci].update(pq=pq, m2=m2, RB_=RB_, HP=HP)
                if R_ == R:
                    nc.gpsimd.dma_start(pq[:], pq_d[t])
                    nc.gpsimd.dma_start(m2[:], mask_d[t])
                else:
                    src = pq_d[t].rearrange(
                        "p (q r b) -> p q r b", q=12, r=R, b=NB
                    )[:, :, roff : roff + R_, :]
                    nc.gpsimd.dma_start(
                        pq.rearrange("p (q r b) -> p q r b", q=12, r=R_, b=NB),
                        src,
                    )
                    msrc = mask_d[t].rearrange(
                        "p (q r b) -> p q r b", q=2, r=R, b=NB
                    )[:, :, roff : roff + R_, :]
                    nc.gpsimd.dma_start(
                        m2.rearrange("p (q r b) -> p q r b", q=2, r=R_, b=NB),
                        msrc,
                    )

            def stage_b(ci):  # diff + squares + bone_valid/den
                s = st[ci]
                pq, m2, RB_, HP = s["pq"], s["m2"], s["RB_"], s["HP"]
                # D = X[j2] - X[j1], both poses, ONE contiguous 2x subtract.
                # Layout [g, c, r, b].
                D = work_pool.tile([P, HP], DT, tag="D")
                nc.vector.tensor_sub(D[:], pq[:, HP:], pq[:, :HP])
                # squares in place; ACT takes ACT_PLANES RB_-sized planes
                # (split per pose so g0 adds unblock earlier), DVE the rest
                split = ACT_PLANES * RB_
                g0p = min(3, ACT_PLANES) * RB_
                nc.scalar.activation(
                    D[:, :g0p], D[:, :g0p],
                    mybir.ActivationFunctionType.Square,
                )
                if g0p < split:
                    nc.scalar.activation(
                        D[:, g0p:split], D[:, g0p:split],
                        mybir.ActivationFunctionType.Square,
                    )
                if split < HP:
                    nc.vector.tensor_mul(D[:, split:], D[:, split:], D[:, split:])
                # bone_valid = mask[J1]*mask[J2]; den = sum(MV) via the
                # TensorEngine ones-matmul, accumulating in PSUM across all
                # chunks.  NOTE: GpSimd elementwise is avoided everywhere —
                # it shares an SBUF port with DVE and measurably inflates
                # every DVE op when active.
                MV = work_pool.tile([P, RB_], DT, tag="MV", bufs=4)
                nc.vector.tensor_mul(MV[:], m2[:, :RB_], m2[:, RB_:])
                half = RB_ // 2
                for h in range(2):
                    nc.tensor.matmul(
                        den_ps[h][:, :half],
                        ones[:],
                        MV[:, h * half : (h + 1) * half],
                        start=False,
                        stop=(ci == NCH - 1),
                        skip_group_check=True,
                    )
                s.update(D=D, MV=MV)

            def stage_c(ci):  # coord sums + sqrt + length diff
                s = st[ci]
                D, RB_ = s["D"], s["RB_"]
                L2 = work_pool.tile([P, 2 * RB_], DT, tag="L2")
                for g in range(2):
                    d0 = 3 * g * RB_
                    lo = g * RB_
                    nc.vector.tensor_add(
                        L2[:, lo : lo + RB_],
                        D[:, d0 : d0 + RB_],
                        D[:, d0 + RB_ : d0 + 2 * RB_],
                    )
                    nc.vector.tensor_add(
                        L2[:, lo : lo + RB_],
                        L2[:, lo : lo + RB_],
                        D[:, d0 + 2 * RB_ : d0 + 3 * RB_],
                    )
                nc.scalar.sqrt(L2[:], L2[:])
                # E = pred_len - ref_len
                nc.vector.tensor_sub(L2[:, :RB_], L2[:, :RB_], L2[:, RB_:])
                s.update(L2=L2)

            def stage_d(ci):  # masked error + num accumulate
                s = st[ci]
                L2, MV, RB_ = s["L2"], s["MV"], s["RB_"]
                ME = L2[:, :RB_]
                nc.vector.tensor_mul(ME, ME, MV[:])
                nc.scalar.activation(
                    ME,
                    ME,
                    mybir.ActivationFunctionType.Square,
                    accum_out=numstrip[:, ci : ci + 1],
                )
                st[ci] = None  # free references

            for it in range(NCH + 3):
                if it < NCH:
                    stage_a(it)
                if 1 <= it < NCH + 1:
                    stage_b(it - 1)
                if 2 <= it < NCH + 2:
                    stage_c(it - 2)
                if it >= 3:
                    stage_d(it - 3)

            acc2 = acc_pool.tile([P, 2], f32)
            nc.vector.memset(acc2[:], 0.0)
            nc.vector.reduce_sum(acc2[:, 0:1], numstrip[:], axis=mybir.AxisListType.X)
            denfin = acc_pool.tile([1, RB], f32)
            nc.vector.tensor_copy(denfin[:, : RB // 2], den_ps[0][:1])
            nc.vector.tensor_copy(denfin[:, RB // 2 :], den_ps[1][:1])
            nc.vector.reduce_sum(acc2[0:1, 1:2], denfin[:], axis=mybir.AxisListType.X)
            nc.gpsimd.dma_start(out_d[:], acc2[:])

    nc.compile()
    return nc


def _get_nc():
    global _COMPILED
    if _COMPILED is None:
        _COMPILED = _build()
    return _COMPILED


def _make_in_maps(pose_3d_pred, pose_3d_ref, valid_mask):
    import concourse.mybir as mybir

    bf16 = mybir.dt.np(mybir.dt.bfloat16)
    BJ = np.concatenate([J1, J2])  # J-major endpoint order (64)

    pred = np.asarray(pose_3d_pred, dtype=np.float32)
    ref = np.asarray(pose_3d_ref, dtype=np.float32)
    mask = np.asarray(valid_mask).astype(np.float32)

    # gather endpoints, cast, and lay out [core,T,P, J,g,c,R,b]
    ga = np.stack([pred[:, BJ, :], ref[:, BJ, :]])  # [g, B, 2J*32, c]
    ga = ga.astype(bf16)
    ga = ga.reshape(2, N_CORES, T, P, R, 2, NB, 3)
    ga = ga.transpose(1, 2, 3, 5, 0, 7, 4, 6)  # -> core,T,P,J,g,c,R,b
    ga = np.ascontiguousarray(ga).reshape(N_CORES, T, P, 2 * 2 * 3 * R * NB)

    mg = mask[:, BJ].astype(bf16)  # [B, 64]
    mg = mg.reshape(N_CORES, T, P, R, 2, NB).transpose(0, 1, 2, 4, 3, 5)
    mg = np.ascontiguousarray(mg).reshape(N_CORES, T, P, 2 * R * NB)

    return [{"pq": ga[c], "mask": mg[c]} for c in range(N_CORES)]


def kernel(pose_3d_pred, pose_3d_ref, valid_mask, _trace=False):
    from concourse.bass_utils import run_bass_kernel_spmd

    nc = _get_nc()
    in_maps = _make_in_maps(pose_3d_pred, pose_3d_ref, valid_mask)
    res = run_bass_kernel_spmd(nc, in_maps, list(range(N_CORES)), trace=_trace)
    num = 0.0
    den = 0.0
    for i in range(N_CORES):
        o = res.results[i]["out"].astype(np.float64)
        num += o[:, 0].sum()
        den += o[:, 1].sum()
    out = np.float32(num / den)
    if _trace:
        return out, res
    return out
